# revision 2
# baseline (speedup 1.0000x reference)
"""Trainium2 Bass kernel for the recurrent STP network (nn_Network_20109036880204).

Strategy: tensor-parallel over the output-neuron dim across 8 NeuronCores.
  - Each core owns a 1024-neuron shard: W_c = Wab[c*1024:(c+1)*1024, :]^T,
    stored fp16 resident in SBUF as 64 K-tiles [128, 1024] (128 KiB/partition).
  - All [B, N] state tensors live in SBUF in "state layout": tile [128, 256]
    with  tile[p, j*32 + b] = state[b, n = c*1024 + j*128 + p].
    (n on partitions -> fast 128-lane elementwise AND the matmul's stationary
    operand y^T [128, 32] is a contiguous free-dim slice.)
  - Per step: y = u'*x'*r (fp16) -> DRAM -> AllGather(8) -> y_full in SBUF ->
    128 matmuls (K=8192 in 64 tiles, N=1024 in 2 PSUM chunks) -> PE transpose
    of the [32, 1024] result back into state layout -> fused DVE update chain.
"""

import sys

for _p in ("/opt/trn_rl_repo", "/root/.axon_site/_ro/trn_rl_repo"):
    if _p not in sys.path:
        sys.path.append(_p)

import numpy as np

import concourse.bass as bass
import concourse.bacc as bacc
import concourse.mybir as mybir
import concourse.tile as tile
from concourse import bass_utils, masks

# problem constants
NCORES = 8
B = 32
N = 8192
NS = N // NCORES          # 1024 neurons per core
P = 128
J = NS // P               # 8 local K-tiles per core
T = N // P                # 64 K-tiles total
F = J * B                 # 256 = free size of a state tile
CHUNK = 512               # matmul moving free dim (one PSUM bank)
NCH = NS // CHUNK         # 2 chunks

DT = 0.01
USE = 0.03
TAU_FAC = 1.0
TAU_REC = 0.25
C1 = DT / TAU_FAC         # 0.01
C0 = DT * USE / TAU_FAC   # 3e-4
A1 = USE * DT             # 3e-4
C2 = DT / TAU_REC         # 0.04

F32 = mybir.dt.float32
F16 = mybir.dt.float16
MULT = mybir.AluOpType.mult
ADD = mybir.AluOpType.add
MAX = mybir.AluOpType.max


def build_program(n_steps: int):
    """Build the SPMD Bass program (identical on all 8 cores)."""
    nc = bacc.Bacc(
        "TRN2",
        target_bir_lowering=False,
        debug=False,
        num_devices=NCORES,
    )

    w_dram = nc.dram_tensor("w", [T, P, NS], F16, kind="ExternalInput")
    sd = {
        nm: nc.dram_tensor(nm, [P, F], F32, kind="ExternalInput")
        for nm in ["r0", "recs0", "u0", "x0", "ff", "es", "ds", "e", "dt"]
    }
    r_out = nc.dram_tensor("r_out", [P, F], F32, kind="ExternalOutput")

    with tile.TileContext(nc) as tc:
        with (
            tc.tile_pool(name="wpool", bufs=1) as wpool,
            tc.tile_pool(name="cpool", bufs=1) as cpool,
            tc.tile_pool(name="spool", bufs=2) as spool,
            tc.tile_pool(name="wk", bufs=2) as wk,
            tc.tile_pool(name="yp", bufs=2) as yp,
            tc.tile_pool(name="pmm", bufs=2, space="PSUM") as pmm,
            tc.tile_pool(name="pT", bufs=2, space="PSUM") as pT,
            tc.tile_pool(name="dp", bufs=2, space="DRAM") as dp,
        ):
            # ---- resident weights: 16 DMAs so they spread across queues ----
            w_sb = wpool.tile([P, T * NS], F16, tag="w")
            TB = 4  # K-tiles per DMA
            for i in range(T // TB):
                dst = w_sb[:, i * TB * NS:(i + 1) * TB * NS].rearrange(
                    "p (t n) -> p t n", t=TB
                )
                src = w_dram[i * TB:(i + 1) * TB, :, :].rearrange("t p n -> p t n")
                nc.sync.dma_start(dst, src)

            # ---- constants / initial state ----
            ff_sb = cpool.tile([P, F], F32, tag="ff")
            es_sb = cpool.tile([P, F], F32, tag="es")
            ds_sb = cpool.tile([P, F], F32, tag="ds")
            e_sb = cpool.tile([P, F], F32, tag="e")
            dt_sb = cpool.tile([P, F], F32, tag="dt")
            ident = cpool.tile([B, B], F32, tag="ident")
            for t_, nm in [(ff_sb, "ff"), (es_sb, "es"), (ds_sb, "ds"),
                           (e_sb, "e"), (dt_sb, "dt")]:
                nc.sync.dma_start(t_[:], sd[nm][:])
            masks.make_identity(nc, ident[:])

            r = spool.tile([P, F], F32, tag="r")
            recS = spool.tile([P, F], F32, tag="recS")
            u0_sb = wk.tile([P, F], F32, tag="u0")
            x0_sb = wk.tile([P, F], F32, tag="x0")
            for t_, nm in [(r, "r0"), (recS, "recs0"), (u0_sb, "u0"),
                           (x0_sb, "x0")]:
                nc.sync.dma_start(t_[:], sd[nm][:])

            V = nc.vector

            # ---- prologue: u1, x1, y0 from initial state ----
            s1 = wk.tile([P, F], F32, tag="t0")
            m = wk.tile([P, F], F32, tag="t1")
            s2 = wk.tile([P, F], F32, tag="t2")
            un = spool.tile([P, F], F32, tag="u")
            V.tensor_scalar(s1[:], u0_sb[:], 1.0 - C1, C0, MULT, ADD)
            V.tensor_mul(m[:], u0_sb[:], r[:])
            V.scalar_tensor_tensor(s2[:], r[:], A1, s1[:], MULT, ADD)
            V.scalar_tensor_tensor(un[:], m[:], -A1, s2[:], MULT, ADD)

            t2p = wk.tile([P, F], F32, tag="t0")
            t3p = wk.tile([P, F], F32, tag="t1")
            s4 = wk.tile([P, F], F32, tag="t2")
            xn = spool.tile([P, F], F32, tag="x")
            V.tensor_mul(t2p[:], x0_sb[:], r[:])
            V.tensor_mul(t3p[:], un[:], t2p[:])
            V.tensor_scalar(s4[:], x0_sb[:], 1.0 - C2, C2, MULT, ADD)
            V.scalar_tensor_tensor(xn[:], t3p[:], -DT, s4[:], MULT, ADD)

            w0 = wk.tile([P, F], F32, tag="t0")
            y16 = yp.tile([P, F], F16, tag="y16")
            V.tensor_mul(w0[:], un[:], xn[:])
            V.tensor_mul(y16[:], w0[:], r[:])

            # ---- main loop (fully unrolled; collectives must be top-level) ----
            for it in range(n_steps):
                last = it == n_steps - 1

                # y -> DRAM (partition-split so it spreads across DMA queues)
                y_dram = dp.tile([P, F], F16, tag="y_dram")
                for qi in range(4):
                    nc.sync.dma_start(
                        y_dram[32 * qi:32 * (qi + 1), :],
                        y16[32 * qi:32 * (qi + 1), :],
                    )
                yall = dp.tile([NCORES, P, F], F16, tag="yall")
                nc.gpsimd.collective_compute(
                    "AllGather",
                    mybir.AluOpType.bypass,
                    replica_groups=[list(range(NCORES))],
                    ins=[y_dram.opt()],
                    outs=[yall.opt()],
                )
                yfull = yp.tile([P, NCORES * F], F16, tag="yfull")
                for c in range(NCORES):
                    nc.sync.dma_start(yfull[:, c * F:(c + 1) * F], yall[c, :, :])

                # precompute (overlaps AG + matmul on DVE)
                A_t = wk.tile([P, F], F32, tag="A")
                B_t = wk.tile([P, F], F32, tag="B")
                C_t = wk.tile([P, F], F32, tag="C")
                D_t = wk.tile([P, F], F32, tag="D")
                rE = wk.tile([P, F], F32, tag="rE")
                V.tensor_scalar(A_t[:], un[:], 1.0 - C1, C0, MULT, ADD)
                V.tensor_scalar(B_t[:], un[:], -A1, A1, MULT, ADD)
                V.tensor_scalar(C_t[:], xn[:], 1.0 - C2, C2, MULT, ADD)
                V.tensor_scalar(D_t[:], xn[:], DT, None, MULT)
                V.tensor_mul(rE[:], r[:], e_sb[:])

                # matmul: 2 PSUM chunks x 64 K-tiles, then transpose to
                # state layout via ACT copy + PE transpose
                mmT = pT.tile([P, F], F32, tag="mmT")
                stage = wk.tile([B, NS], F32, tag="stage")
                for ch in range(NCH):
                    pm = pmm.tile([B, CHUNK], F32, tag=f"mm{ch}")
                    for t in range(T):
                        nc.tensor.matmul(
                            pm[:],
                            lhsT=yfull[:, t * B:(t + 1) * B],
                            rhs=w_sb[:, t * NS + ch * CHUNK:
                                     t * NS + (ch + 1) * CHUNK],
                            start=(t == 0),
                            stop=(t == T - 1),
                        )
                    nc.scalar.copy(stage[:, ch * CHUNK:(ch + 1) * CHUNK], pm[:])
                    for jj in range(CHUNK // P):
                        j = (ch * CHUNK) // P + jj
                        nc.tensor.transpose(
                            mmT[:, j * B:(j + 1) * B],
                            stage[:, j * P:(j + 1) * P],
                            ident[:],
                        )

                # critical chain: rec', r', then next (u, x, y)
                rec_new = spool.tile([P, F], F32, tag="recfull")
                tmp = wk.tile([P, F], F32, tag="t0")
                V.tensor_mul(tmp[:], mmT[:], ds_sb[:])
                V.tensor_add(rec_new[:], tmp[:], recS[:])
                h = wk.tile([P, F], F32, tag="t1")
                V.tensor_add(h[:], rec_new[:], ff_sb[:])
                drelu = wk.tile([P, F], F32, tag="t2")
                V.scalar_tensor_tensor(drelu[:], h[:], 0.0, dt_sb[:], MAX, MULT)
                r_new = spool.tile([P, F], F32, tag="r")
                V.tensor_add(r_new[:], drelu[:], rE[:])

                if not last:
                    recS_new = spool.tile([P, F], F32, tag="recS")
                    V.tensor_mul(recS_new[:], rec_new[:], es_sb[:])
                    m1 = wk.tile([P, F], F32, tag="t0")
                    q = spool.tile([P, F], F32, tag="u")
                    tt2 = wk.tile([P, F], F32, tag="t1")
                    s_ = wk.tile([P, F], F32, tag="t3")
                    v = spool.tile([P, F], F32, tag="x")
                    y_new = yp.tile([P, F], F16, tag="y16")
                    V.tensor_mul(m1[:], B_t[:], r_new[:])
                    V.tensor_add(q[:], m1[:], A_t[:])
                    V.tensor_mul(tt2[:], r_new[:], q[:])
                    V.tensor_mul(s_[:], D_t[:], tt2[:])
                    V.scalar_tensor_tensor(v[:], s_[:], -1.0, C_t[:], MULT, ADD)
                    V.tensor_mul(y_new[:], tt2[:], v[:])
                    un, xn, y16, recS = q, v, y_new, recS_new

                r = r_new

            # ---- epilogue ----
            for qi in range(4):
                nc.sync.dma_start(
                    r_out[32 * qi:32 * (qi + 1), :],
                    r[32 * qi:32 * (qi + 1), :],
                )

    nc.compile()
    return nc


# ---------------------------------------------------------------------------
# host-side data marshalling
# ---------------------------------------------------------------------------

def _shard_state(v, c):
    """[B, N] float array -> core c state tile [128, 256] (f32)."""
    vs = np.asarray(v, np.float32)[:, c * NS:(c + 1) * NS]      # [32, 1024]
    return np.ascontiguousarray(
        vs.reshape(B, J, P).transpose(2, 1, 0).reshape(P, F)
    )


def _shard_vec(v, c):
    """[N] float vector -> replicated core c tile [128, 256] (f32)."""
    vs = np.asarray(v, np.float32)[c * NS:(c + 1) * NS].reshape(J, P)  # [j, p]
    t = vs.T[:, :, None]                                        # [p, j, 1]
    return np.ascontiguousarray(np.broadcast_to(t, (P, J, B)).reshape(P, F))


def _shard_w(Wab, c):
    """Wab [N, N] -> core c weight tiles [64, 128, 1024] fp16.

    w[t, p, n] = Wab[c*1024 + n, t*128 + p]
    """
    wt = np.asarray(Wab, np.float32)[c * NS:(c + 1) * NS, :].T  # [8192, 1024]
    return np.ascontiguousarray(wt.astype(np.float16).reshape(T, P, NS))


def _unshard_out(tiles):
    """list of 8 [128, 256] tiles -> [32, 8192] f32."""
    out = np.empty((B, N), np.float32)
    for c, tl in enumerate(tiles):
        out[:, c * NS:(c + 1) * NS] = (
            np.asarray(tl, np.float32).reshape(P, J, B).transpose(2, 1, 0)
            .reshape(B, NS)
        )
    return out


def make_in_maps(rates, rec_input, ff_input, Wab, u_stp, x_stp,
                 exp_dt_tau, dt_tau, exp_dt_tau_syn, dt_tau_syn):
    recs_full = (np.asarray(exp_dt_tau_syn, np.float32)[None, :]
                 * np.asarray(rec_input, np.float32))
    in_maps = []
    for c in range(NCORES):
        in_maps.append({
            "w": _shard_w(Wab, c),
            "r0": _shard_state(rates, c),
            "recs0": _shard_state(recs_full, c),
            "u0": _shard_state(u_stp, c),
            "x0": _shard_state(x_stp, c),
            "ff": _shard_state(ff_input, c),
            "es": _shard_vec(exp_dt_tau_syn, c),
            "ds": _shard_vec(dt_tau_syn, c),
            "e": _shard_vec(exp_dt_tau, c),
            "dt": _shard_vec(dt_tau, c),
        })
    return in_maps


_PROGRAM_CACHE = {}


def _get_program(n_steps):
    if n_steps not in _PROGRAM_CACHE:
        _PROGRAM_CACHE[n_steps] = build_program(n_steps)
    return _PROGRAM_CACHE[n_steps]


def run(trace=False, tmpdir=None, **inputs):
    n_steps = int(inputs.pop("n_steps"))
    nc = _get_program(n_steps)
    in_maps = make_in_maps(**inputs)
    res = bass_utils.run_bass_kernel_spmd(
        nc, in_maps, core_ids=list(range(NCORES)), trace=trace, tmpdir=tmpdir
    )
    out = _unshard_out([m["r_out"] for m in res.results])
    return out, res


def kernel(**inputs):
    out, _ = run(**inputs)
    return out


# revision 12
# speedup vs baseline: 1.2467x; 1.2467x over previous
"""Trainium2 Bass kernel for the recurrent STP network (nn_Network_20109036880204).

Strategy: tensor-parallel over the output-neuron dim across 8 NeuronCores.
  - Each core owns a 1024-neuron shard: W_c = Wab[c*1024:(c+1)*1024, :]^T,
    stored fp16 resident in SBUF as 64 K-tiles [128, 1024] (128 KiB/partition).
  - All [B, N] state tensors live in SBUF in "state layout": tile [128, 256]
    with  tile[p, j*32 + b] = state[b, n = c*1024 + j*128 + p].
    (n on partitions -> fast 128-lane elementwise AND the matmul's stationary
    operand y^T [128, 32] is a contiguous free-dim slice.)
  - Per step: y = u'*x'*r (fp16) -> DRAM -> AllGather(8) -> y_full in SBUF ->
    128 matmuls (K=8192 in 64 tiles, N=1024 in 2 PSUM chunks) -> PE transpose
    of the [32, 1024] result back into state layout -> fused DVE update chain.
"""

import sys

for _p in ("/opt/trn_rl_repo", "/root/.axon_site/_ro/trn_rl_repo"):
    if _p not in sys.path:
        sys.path.append(_p)

import numpy as np

import concourse.bass as bass
import concourse.bacc as bacc
import concourse.mybir as mybir
import concourse.tile as tile
from concourse import bass_utils, masks

# problem constants
NCORES = 8
B = 32
N = 8192
NS = N // NCORES          # 1024 neurons per core
P = 128
J = NS // P               # 8 local K-tiles per core
T = N // P                # 64 K-tiles total
F = J * B                 # 256 = free size of a state tile
CHUNK = 512               # matmul moving free dim (one PSUM bank)
NCH = NS // CHUNK         # 2 chunks

DT = 0.01
USE = 0.03
TAU_FAC = 1.0
TAU_REC = 0.25
C1 = DT / TAU_FAC         # 0.01
C0 = DT * USE / TAU_FAC   # 3e-4
A1 = USE * DT             # 3e-4
C2 = DT / TAU_REC         # 0.04

F32 = mybir.dt.float32
F16 = mybir.dt.float16
MULT = mybir.AluOpType.mult
ADD = mybir.AluOpType.add
MAX = mybir.AluOpType.max


HF = F // 2          # 128 = free width of one half of a state tile
A_TILES = [t for t in range(T) if t % (2 * J // 2) < J // 2]  # t%8 < 4
B_TILES = [t for t in range(T) if t % (2 * J // 2) >= J // 2]


def build_program(n_steps: int, pair_lhst: bool = False):
    """Build the SPMD Bass program (identical on all 8 cores).

    Two-half pipeline: each core's y shard is split into half A (j=0..3)
    and half B (j=4..7); each half is all-gathered separately so AG_A can
    fly while the tail of the matmul still runs, and the next step's
    matmul consumes A-sourced K-tiles first.
    """
    nc = bacc.Bacc(
        "TRN2",
        target_bir_lowering=False,
        debug=False,
        num_devices=NCORES,
    )

    w_dram = nc.dram_tensor("w", [T, P, NS], F16, kind="ExternalInput")
    sd = {
        nm: nc.dram_tensor(nm, [P, F], F32, kind="ExternalInput")
        for nm in ["r0", "recs0", "u0", "x0", "ff", "es", "ds", "e", "dt"]
    }
    r_out = nc.dram_tensor("r_out", [P, F], F32, kind="ExternalOutput")

    with tile.TileContext(nc) as tc:
        with (
            tc.tile_pool(name="wpool", bufs=1) as wpool,
            tc.tile_pool(name="cpool", bufs=1) as cpool,
            tc.tile_pool(name="spool", bufs=2) as spool,
            tc.tile_pool(name="wk", bufs=2) as wk,
            tc.tile_pool(name="yp", bufs=2) as yp,
            tc.tile_pool(name="pmm", bufs=2, space="PSUM") as pmm,
            tc.tile_pool(name="pT", bufs=2, space="PSUM") as pT,
            tc.tile_pool(name="dp", bufs=3, space="DRAM") as dp,
        ):
            # ---- resident weights: 16 DMAs so they spread across queues ----
            w_sb = wpool.tile([P, T * NS], F16, tag="w")
            TB = 4  # K-tiles per DMA
            for i in range(T // TB):
                dst = w_sb[:, i * TB * NS:(i + 1) * TB * NS].rearrange(
                    "p (t n) -> p t n", t=TB
                )
                src = w_dram[i * TB:(i + 1) * TB, :, :].rearrange("t p n -> p t n")
                nc.sync.dma_start(dst, src)

            # ---- constants / initial state ----
            ff_sb = cpool.tile([P, F], F32, tag="ff")
            es_sb = cpool.tile([P, F], F32, tag="es")
            ds_sb = cpool.tile([P, F], F32, tag="ds")
            e_sb = cpool.tile([P, F], F32, tag="e")
            dt_sb = cpool.tile([P, F], F32, tag="dt")
            ident = cpool.tile([B, B], F32, tag="ident")
            for t_, nm in [(ff_sb, "ff"), (es_sb, "es"), (ds_sb, "ds"),
                           (e_sb, "e"), (dt_sb, "dt")]:
                nc.sync.dma_start(t_[:], sd[nm][:])
            masks.make_identity(nc, ident[:])

            r = spool.tile([P, F], F32, tag="r")
            recS = spool.tile([P, F], F32, tag="recS")
            u0_sb = wk.tile([P, F], F32, tag="u0", bufs=1)
            x0_sb = wk.tile([P, F], F32, tag="x0", bufs=1)
            for t_, nm in [(r, "r0"), (recS, "recs0"), (u0_sb, "u0"),
                           (x0_sb, "x0")]:
                nc.sync.dma_start(t_[:], sd[nm][:])

            V = nc.vector

            # ---- prologue: u1, x1, y0 from initial state ----
            s1 = wk.tile([P, F], F32, tag="t0", bufs=1)
            m = wk.tile([P, F], F32, tag="t1", bufs=1)
            s2 = wk.tile([P, F], F32, tag="t2", bufs=1)
            un = spool.tile([P, F], F32, tag="u")
            V.tensor_scalar(s1[:], u0_sb[:], 1.0 - C1, C0, MULT, ADD)
            V.tensor_mul(m[:], u0_sb[:], r[:])
            V.scalar_tensor_tensor(s2[:], r[:], A1, s1[:], MULT, ADD)
            V.scalar_tensor_tensor(un[:], m[:], -A1, s2[:], MULT, ADD)

            t2p = wk.tile([P, F], F32, tag="t3", bufs=1)
            t3p = wk.tile([P, F], F32, tag="t4", bufs=1)
            s4 = wk.tile([P, F], F32, tag="t5", bufs=1)
            xn = spool.tile([P, F], F32, tag="x")
            V.tensor_mul(t2p[:], x0_sb[:], r[:])
            V.tensor_mul(t3p[:], un[:], t2p[:])
            V.tensor_scalar(s4[:], x0_sb[:], 1.0 - C2, C2, MULT, ADD)
            V.scalar_tensor_tensor(xn[:], t3p[:], -DT, s4[:], MULT, ADD)

            w0 = wk.tile([P, F], F32, tag="t6", bufs=1)
            yh = {}
            V.tensor_mul(w0[:], un[:], xn[:])
            for hf, sl in (("A", slice(0, HF)), ("B", slice(HF, F))):
                yh[hf] = yp.tile([P, HF], F16, tag=f"y{hf}", name=f"y{hf}_pro")
                V.tensor_mul(yh[hf][:], w0[:, sl], r[:, sl])

            ag_counter = [0]

            def launch_ag(hf, ytile):
                """store y-half to DRAM, AllGather, DMA gathered tiles back."""
                k = ag_counter[0] = ag_counter[0] + 1
                ydr = dp.tile([P, HF], F16, tag=f"ydr{hf}", name=f"ydr{hf}_{k}")
                for qi in range(2):
                    nc.sync.dma_start(
                        ydr[64 * qi:64 * (qi + 1), :],
                        ytile[64 * qi:64 * (qi + 1), :],
                    )
                yall = dp.tile([NCORES, P, HF], F16, tag=f"yall{hf}",
                               name=f"yall{hf}_{k}")
                nc.gpsimd.collective_compute(
                    "AllGather",
                    mybir.AluOpType.bypass,
                    replica_groups=[list(range(NCORES))],
                    ins=[ydr.opt()],
                    outs=[yall.opt()],
                )
                yfull = yp.tile([P, NCORES * HF], F16, tag=f"yfull{hf}",
                                name=f"yfull{hf}_{k}")
                for c in range(NCORES):
                    nc.sync.dma_start(
                        yfull[:, c * HF:(c + 1) * HF], yall[c, :, :]
                    )
                return yfull

            yfullA = launch_ag("A", yh["A"])
            yfullB = launch_ag("B", yh["B"])

            def lhst_ap(yfA, yfB, t):
                c, j = divmod(t, J)
                if j < J // 2:
                    return yfA[:, c * HF + j * B:c * HF + (j + 1) * B]
                return yfB[:, c * HF + (j - J // 2) * B:c * HF + (j - J // 2 + 1) * B]

            # ---- main loop ----
            for it in range(n_steps):
                last = it == n_steps - 1

                # precompute (overlaps AG + matmul on DVE)
                A_t = wk.tile([P, F], F32, tag="A", bufs=1)
                B_t = wk.tile([P, F], F32, tag="B", bufs=1)
                C_t = wk.tile([P, F], F32, tag="C", bufs=1)
                D_t = wk.tile([P, F], F32, tag="D", bufs=1)
                rE = wk.tile([P, F], F32, tag="rE", bufs=1)
                if not last:
                    V.tensor_scalar(A_t[:], un[:], 1.0 - C1, C0, MULT, ADD)
                    V.tensor_scalar(B_t[:], un[:], -A1, A1, MULT, ADD)
                    V.tensor_scalar(C_t[:], xn[:], 1.0 - C2, C2, MULT, ADD)
                    V.tensor_scalar(D_t[:], xn[:], DT, None, MULT)
                V.tensor_mul(rE[:], r[:], e_sb[:])

                # matmul order: [c0:A] [c1:A] [c0:B] [+A-transposes] [c1:B]
                # so chunk 0 finishes 3/4 of the way in and half A's
                # transpose + ew chain + AllGather all fly under c1:B.
                pm = [pmm.tile([B, CHUNK], F32, tag=f"mm{ch}", name=f"pm{ch}_{it}")
                      for ch in range(NCH)]
                nmm = {0: 0, 1: 0}

                def emit_group(ch, tiles):
                    for t in tiles:
                        nc.tensor.matmul(
                            pm[ch][:],
                            lhsT=lhst_ap(yfullA, yfullB, t),
                            rhs=w_sb[:, t * NS + ch * CHUNK:
                                     t * NS + (ch + 1) * CHUNK],
                            start=(nmm[ch] == 0),
                            stop=(nmm[ch] == T - 1),
                        )
                        nmm[ch] += 1

                def transpose_half(hf, pm_ch):
                    """PSUM chunk [32, 512] -> state-layout PSUM [128, 128].

                    Sliced 128-col copies so each PE transpose only waits on
                    its own small ACT copy (~0.2us), not the whole chunk.
                    """
                    mmT_ = pT.tile([P, HF], F32, tag=f"mmT{hf}")
                    stage = wk.tile([B, CHUNK], F32, tag=f"stage{hf}", bufs=1)
                    for jj in range(4):
                        nc.scalar.copy(stage[:, jj * P:(jj + 1) * P],
                                       pm_ch[:, jj * P:(jj + 1) * P])
                        nc.tensor.transpose(
                            mmT_[:, jj * B:(jj + 1) * B],
                            stage[:, jj * P:(jj + 1) * P],
                            ident[:],
                        )
                    return mmT_

                emit_group(0, A_TILES)
                emit_group(1, A_TILES)
                emit_group(0, B_TILES)
                mmTA = transpose_half("A", pm[0])
                emit_group(1, B_TILES)

                # names for per-half state pieces of this iteration
                rec_new = spool.tile([P, F], F32, tag="recfull")
                r_new = spool.tile([P, F], F32, tag="r")
                recS_new = spool.tile([P, F], F32, tag="recS")
                q = spool.tile([P, F], F32, tag="u")
                v = spool.tile([P, F], F32, tag="x")
                newy = {"A": yp.tile([P, HF], F16, tag="yA", name=f"yA_{it}"),
                        "B": yp.tile([P, HF], F16, tag="yB", name=f"yB_{it}")}

                def ew_half(hf, mmT_half):
                    sl = slice(0, HF) if hf == "A" else slice(HF, F)
                    tmp = wk.tile([P, HF], F32, tag=f"w0{hf}", bufs=1)
                    V.tensor_mul(tmp[:], mmT_half[:], ds_sb[:, sl])
                    V.tensor_add(rec_new[:, sl], tmp[:], recS[:, sl])
                    h_ = wk.tile([P, HF], F32, tag=f"w1{hf}", bufs=1)
                    V.tensor_add(h_[:], rec_new[:, sl], ff_sb[:, sl])
                    dr_ = wk.tile([P, HF], F32, tag=f"w2{hf}", bufs=1)
                    V.scalar_tensor_tensor(dr_[:], h_[:], 0.0, dt_sb[:, sl],
                                           MAX, MULT)
                    V.tensor_add(r_new[:, sl], dr_[:], rE[:, sl])
                    if last:
                        return None
                    V.tensor_mul(recS_new[:, sl], rec_new[:, sl], es_sb[:, sl])
                    m1_ = wk.tile([P, HF], F32, tag=f"w3{hf}", bufs=1)
                    V.tensor_mul(m1_[:], B_t[:, sl], r_new[:, sl])
                    V.tensor_add(q[:, sl], m1_[:], A_t[:, sl])
                    tt_ = wk.tile([P, HF], F32, tag=f"w4{hf}", bufs=1)
                    V.tensor_mul(tt_[:], r_new[:, sl], q[:, sl])
                    s2_ = wk.tile([P, HF], F32, tag=f"w5{hf}", bufs=1)
                    V.tensor_mul(s2_[:], D_t[:, sl], tt_[:])
                    V.scalar_tensor_tensor(v[:, sl], s2_[:], -1.0, C_t[:, sl],
                                           MULT, ADD)
                    ynew = newy[hf]
                    V.tensor_mul(ynew[:], tt_[:], v[:, sl])
                    return ynew

                yA_next = ew_half("A", mmTA)
                if not last:
                    nextA = launch_ag("A", yA_next)

                # chunk 1 complete -> half B
                mmTB = transpose_half("B", pm[1])
                yB_next = ew_half("B", mmTB)
                if not last:
                    nextB = launch_ag("B", yB_next)
                    yfullA, yfullB = nextA, nextB
                    un, xn, recS = q, v, recS_new
                r = r_new

            # ---- epilogue ----
            for qi in range(4):
                nc.sync.dma_start(
                    r_out[32 * qi:32 * (qi + 1), :],
                    r[32 * qi:32 * (qi + 1), :],
                )

    nc.compile()
    return nc


# ---------------------------------------------------------------------------
# host-side data marshalling
# ---------------------------------------------------------------------------

def _shard_state(v, c):
    """[B, N] float array -> core c state tile [128, 256] (f32)."""
    vs = np.asarray(v, np.float32)[:, c * NS:(c + 1) * NS]      # [32, 1024]
    return np.ascontiguousarray(
        vs.reshape(B, J, P).transpose(2, 1, 0).reshape(P, F)
    )


def _shard_vec(v, c):
    """[N] float vector -> replicated core c tile [128, 256] (f32)."""
    vs = np.asarray(v, np.float32)[c * NS:(c + 1) * NS].reshape(J, P)  # [j, p]
    t = vs.T[:, :, None]                                        # [p, j, 1]
    return np.ascontiguousarray(np.broadcast_to(t, (P, J, B)).reshape(P, F))


def _shard_w(Wab, c):
    """Wab [N, N] -> core c weight tiles [64, 128, 1024] fp16.

    w[t, p, n] = Wab[c*1024 + n, t*128 + p]
    """
    wt = np.asarray(Wab, np.float32)[c * NS:(c + 1) * NS, :].T  # [8192, 1024]
    return np.ascontiguousarray(wt.astype(np.float16).reshape(T, P, NS))


def _unshard_out(tiles):
    """list of 8 [128, 256] tiles -> [32, 8192] f32."""
    out = np.empty((B, N), np.float32)
    for c, tl in enumerate(tiles):
        out[:, c * NS:(c + 1) * NS] = (
            np.asarray(tl, np.float32).reshape(P, J, B).transpose(2, 1, 0)
            .reshape(B, NS)
        )
    return out


def make_in_maps(rates, rec_input, ff_input, Wab, u_stp, x_stp,
                 exp_dt_tau, dt_tau, exp_dt_tau_syn, dt_tau_syn):
    recs_full = (np.asarray(exp_dt_tau_syn, np.float32)[None, :]
                 * np.asarray(rec_input, np.float32))
    in_maps = []
    for c in range(NCORES):
        in_maps.append({
            "w": _shard_w(Wab, c),
            "r0": _shard_state(rates, c),
            "recs0": _shard_state(recs_full, c),
            "u0": _shard_state(u_stp, c),
            "x0": _shard_state(x_stp, c),
            "ff": _shard_state(ff_input, c),
            "es": _shard_vec(exp_dt_tau_syn, c),
            "ds": _shard_vec(dt_tau_syn, c),
            "e": _shard_vec(exp_dt_tau, c),
            "dt": _shard_vec(dt_tau, c),
        })
    return in_maps


_PROGRAM_CACHE = {}


def _get_program(n_steps):
    if n_steps not in _PROGRAM_CACHE:
        _PROGRAM_CACHE[n_steps] = build_program(n_steps)
    return _PROGRAM_CACHE[n_steps]


def run(trace=False, tmpdir=None, **inputs):
    n_steps = int(inputs.pop("n_steps"))
    nc = _get_program(n_steps)
    in_maps = make_in_maps(**inputs)
    res = bass_utils.run_bass_kernel_spmd(
        nc, in_maps, core_ids=list(range(NCORES)), trace=trace, tmpdir=tmpdir
    )
    out = _unshard_out([m["r_out"] for m in res.results])
    return out, res


def kernel(**inputs):
    out, _ = run(**inputs)
    return out
